# revision 58
# baseline (speedup 1.0000x reference)
"""Trainium2 Bass kernel for the neural-renderer loss model.

Pixels are sharded 16 image rows per core across 8 cores.  Each core's
2048 pixels are processed as 16 blocks of 128 pixels (2 rows x 64 cols).

Device pipeline (v3):
  1. Host folds camera + edge-function setup into per-(block,slot) affine
     coefficients over a *constant* pixel basis (1, xs0[j], r), where
     xs0 = x of the left half-row and r = row-within-block.  One tiny
     [3,128] stationary operand serves every block's matmul.
  2. Per 8-block window: 8 matmuls -> one PSUM [128,2048] grid; one
     grouped negated max-reduce (4 cols/slot -> key) -> nk; one grouped
     max-reduce -> per-block maxes; one max_index over the window ->
     winning slot; one indirect DMA gathers the winner's slot row
     (bf16 tanh'd texture cube, c-major) + a second for fp32 barycentric
     coefficients.
  3. Tail per window: winner barycentrics from the same folded coeffs
     (2 fused scalar_tensor_tensor ops), clip/renorm, tent weights
     tent(k) = relu(1-|3q-k|) built for all 3 axes in 5 ops, separable
     outer-product weights w64, one 2x bf16 multiply against the cube,
     in-place halving tree for the sum, hit mask, diff vs reference.
  4. Squared-error accumulate (ScalarE) + ones-matmul partition reduce;
     host sums the 8 per-core scalars.
"""
import numpy as np
import ml_dtypes

H = W = 128
TS = 4
F = 2560
DIST, ELEV, AZIM = 2.732, 0.0, 90.0
NCORES = 8
TPC = H // NCORES            # image rows per core
KSCALE = 1e20
DSHIFT = 4.0                 # small shift keeps depth positive yet precise
HIT_THRESH = 1e6

BR, BC = 2, 64               # block shape (rows x cols), 128 px/block
NBLK = TPC * 128 // (BR * BC)   # 16 blocks per core
CAP = 64                     # face slots per block
NWIN = 2                     # winner-search windows (8 blocks each)
WB = NBLK // NWIN            # blocks per window
CUBE = 192                   # bf16 cube row (c-major 3*4*4*4)
CROW = 12                    # fp32 coef row (9 used + pad)
SROW = 256                   # combined slot row in bf16 units: cube + 9*fp32
                             # + pad to 512B (dma_gather elem alignment)

_prog_cache = {}


def _geom(vertices, faces):
    v64 = np.asarray(vertices[0], np.float64)
    el, az = np.deg2rad(ELEV), np.deg2rad(AZIM)
    eye = DIST * np.array(
        [np.cos(el) * np.sin(az), np.sin(el), -np.cos(el) * np.cos(az)]
    )
    up = np.array([0.0, 1.0, 0.0])
    z = -eye / np.linalg.norm(eye)
    x = np.cross(up, z); x = x / np.linalg.norm(x)
    y = np.cross(z, x)
    R = np.stack([x, y, z])
    vc = (v64 - eye) @ R.T
    tri = vc[np.asarray(faces[0])]               # [F,3,3]
    a, b, c = tri[:, 0], tri[:, 1], tri[:, 2]
    area = (b[:, 0] - a[:, 0]) * (c[:, 1] - a[:, 1]) - \
           (b[:, 1] - a[:, 1]) * (c[:, 0] - a[:, 0])
    sa = np.where(np.abs(area) < 1e-8, 1e-8, area)
    valid = np.abs(area) >= 1e-8

    def edge_coeffs(p, q):
        # edge(p,q,pt) = (qx-px)(pty-py) - (qy-py)(ptx-px) = A + B*ptx + C*pty
        A = p[:, 0] * q[:, 1] - p[:, 1] * q[:, 0]
        B = -(q[:, 1] - p[:, 1])
        C = q[:, 0] - p[:, 0]
        return np.stack([A, B, C])               # [3,F]

    w0c = edge_coeffs(b, c) / sa
    w1c = edge_coeffs(c, a) / sa
    w2c = edge_coeffs(a, b) / sa
    z3 = tri[:, :, 2]
    Dc = w0c * z3[:, 0] + w1c * z3[:, 1] + w2c * z3[:, 2]
    p2x = np.stack([a[:, 0], b[:, 0], c[:, 0]])
    p2y = np.stack([a[:, 1], b[:, 1], c[:, 1]])
    return dict(w0c=w0c, w1c=w1c, w2c=w2c, Dc=Dc, valid=valid,
                bbx=(p2x.min(0), p2x.max(0)), bby=(p2y.min(0), p2y.max(0)))


def _bin_faces(geom):
    """Per-(core, block) conservative face lists. None on CAP overflow."""
    xs = ((np.arange(W, dtype=np.float64) + 0.5) / W * 2.0 - 1.0)
    ys = (1.0 - (np.arange(H, dtype=np.float64) + 0.5) / H * 2.0)
    wcs = [geom["w0c"], geom["w1c"], geom["w2c"]]
    valid = geom["valid"]
    nbr, nbc = H // BR, W // BC
    lists = np.full((NCORES, NBLK, CAP), F, np.int64)   # pad = poison face F
    for bi in range(nbr):
        rcy = ys[bi * BR:(bi + 1) * BR]
        cy = (rcy[0] + rcy[-1]) / 2; hy = abs(rcy[-1] - rcy[0]) / 2
        for bj in range(nbc):
            rcx = xs[bj * BC:(bj + 1) * BC]
            cx = (rcx[0] + rcx[-1]) / 2; hx = (rcx[-1] - rcx[0]) / 2
            ok = valid.copy()
            bbx, bby = geom["bbx"], geom["bby"]
            ok &= (bbx[0] <= cx + hx + 1e-6) & (bbx[1] >= cx - hx - 1e-6)
            ok &= (bby[0] <= cy + hy + 1e-6) & (bby[1] >= cy - hy - 1e-6)
            for e in range(3):
                A, B, C = wcs[e][0], wcs[e][1], wcs[e][2]
                wmax = A + B * cx + C * cy + np.abs(B) * hx + np.abs(C) * hy
                eps = 1e-5 * (np.abs(A) + np.abs(B) + np.abs(C))
                ok &= (wmax + eps) >= 0
            idx = np.nonzero(ok)[0]
            if idx.size > CAP:
                # refine with the exact pixel-center test (+ fp slack)
                px = xs[bj * BC:(bj + 1) * BC]
                py = ys[bi * BR:(bi + 1) * BR]
                PY, PX = np.meshgrid(py, px, indexing="ij")
                P0, P1 = PX.ravel()[None, :], PY.ravel()[None, :]
                ins = np.ones((idx.size, BR * BC), bool)
                for e in range(3):
                    A = wcs[e][0][idx]; B = wcs[e][1][idx]; C = wcs[e][2][idx]
                    eps = 1e-5 * (np.abs(A) + np.abs(B) + np.abs(C))
                    w = A[:, None] + B[:, None] * P0 + C[:, None] * P1
                    ins &= (w + eps[:, None]) >= 0
                idx = idx[ins.any(1)]
                if idx.size > CAP:
                    return None
            core = (bi * BR) // TPC
            blkrow = bi - core * (TPC // BR)
            t = blkrow * nbc + bj
            lists[core, t, :idx.size] = idx
    return lists


def _build_binned(loop_n=None, probes=False, ablate=None):
    """Binned program v3. loop_n wraps the body in a hardware loop.

    ablate: "raster" stops after winner selection (loss from nkmax);
    "gather" additionally runs the slot gathers; None = full."""
    from contextlib import ExitStack
    import concourse.bacc as bacc
    import concourse.tile as tile
    from concourse import mybir
    from concourse.bass import IndirectOffsetOnAxis
    from concourse._compat import axon_active

    fp32 = mybir.dt.float32
    fp32r = mybir.dt.float32r
    bf16 = mybir.dt.bfloat16
    u32 = mybir.dt.uint32
    i16 = mybir.dt.int16
    AL = mybir.AluOpType
    nc = bacc.Bacc(
        "TRN2",
        target_bir_lowering=False,
        debug=not axon_active(),
        num_devices=NCORES,
    )

    GCOLS = CAP * 4                       # grid cols per block
    rcb_in = nc.dram_tensor("rcb", [99, NBLK * GCOLS], fp32r,
                            kind="ExternalInput").ap()
    ctab = nc.dram_tensor("ctab", [NBLK * CAP, SROW], bf16,
                          kind="ExternalInput").ap()
    pb_in = nc.dram_tensor("pb", [99, 128], fp32r,
                           kind="ExternalInput").ap()
    xs0_in = nc.dram_tensor("xs0", [128, 1], fp32, kind="ExternalInput").ap()
    rvec_in = nc.dram_tensor("rvec", [128, 1], fp32, kind="ExternalInput").ap()
    kk_in = nc.dram_tensor("kk", [128, NBLK * 12], fp32,
                           kind="ExternalInput").ap()
    selm_in = nc.dram_tensor("selm", [128, WB * 128], mybir.dt.float16,
                             kind="ExternalInput").ap()
    refsl = nc.dram_tensor("refsl", [128, NBLK * 3], fp32,
                           kind="ExternalInput").ap()
    lossp = nc.dram_tensor("lossp", [1, 1], fp32, kind="ExternalOutput").ap()
    if probes:
        p_nkmax = nc.dram_tensor("p_nkmax", [128, NBLK], fp32,
                                 kind="ExternalOutput").ap()
        p_mi8 = nc.dram_tensor("p_mi8", [128, NBLK], u32,
                               kind="ExternalOutput").ap()
        p_gt = nc.dram_tensor("p_gt", [128, NBLK * SROW], bf16,
                              kind="ExternalOutput").ap()
        p_diff = nc.dram_tensor("p_diff", [128, NBLK * 3], fp32,
                                kind="ExternalOutput").ap()
        p_acc = nc.dram_tensor("p_acc", [128, 1], fp32,
                               kind="ExternalOutput").ap()
        p_nk = nc.dram_tensor("p_nk", [128, NBLK * CAP], fp32,
                              kind="ExternalOutput").ap()

    with tile.TileContext(nc) as tc, ExitStack() as ctx:
        const = ctx.enter_context(tc.tile_pool(name="const", bufs=1))
        sb = ctx.enter_context(tc.tile_pool(name="sb", bufs=2))
        sm = ctx.enter_context(tc.tile_pool(name="sm", bufs=2))
        ps = ctx.enter_context(tc.tile_pool(name="ps", bufs=2, space="PSUM"))
        dr = ctx.enter_context(tc.tile_pool(name="dr", bufs=1, space="DRAM"))

        if loop_n is not None:
            ctx.enter_context(tc.For_i(0, loop_n, 1))

        # ---- inputs: rcb in 8 chunks (one per quad-matmul) on 2 HWDGE
        # queues; small consts behind them ----
        pb_t = const.tile([99, 128], fp32r, tag="pb")
        nc.sync.dma_start(out=pb_t[:], in_=pb_in[:])
        rcb_t = const.tile([99, NBLK * GCOLS], fp32r, tag="rcb")
        CH = NBLK * GCOLS // 8
        for q in range(8):
            eng = nc.sync if q % 2 == 0 else nc.scalar
            eng.dma_start(out=rcb_t[:, q * CH:(q + 1) * CH],
                          in_=rcb_in[:, q * CH:(q + 1) * CH])
        xs0 = const.tile([128, 1], fp32, tag="xs0")
        nc.scalar.dma_start(out=xs0[:], in_=xs0_in[:])
        rvec = const.tile([128, 1], fp32, tag="rvec")
        nc.scalar.dma_start(out=rvec[:], in_=rvec_in[:])
        kk = const.tile([128, NBLK * 3, 4], fp32, tag="kk")
        nc.scalar.dma_start(out=kk[:], in_=kk_in[:].rearrange(
            "p (t k) -> p t k", k=4))
        rs = const.tile([128, NBLK, 3], fp32, tag="rs")
        nc.sync.dma_start(out=rs[:], in_=refsl[:].rearrange(
            "p (t c) -> p t c", c=3))
        selm = const.tile([128, WB * 128], mybir.dt.float16, tag="selm")
        nc.scalar.dma_start(out=selm[:], in_=selm_in[:])

        # ---- persistent result tiles ----
        nk = const.tile([128, NBLK * CAP], fp32, tag="nk")
        nkmax = const.tile([128, NBLK], fp32, tag="nkmax")
        mi8 = const.tile([128, NBLK], u32, tag="mi8")
        mif = const.tile([128, NBLK], mybir.dt.float16, tag="mif")
        idxw = const.tile([128, NBLK * WB], i16, tag="idxw")
        gt = const.tile([128, NBLK, SROW], bf16, tag="gt")
        diff = const.tile([128, NBLK, 3], fp32, tag="diff")

        # ---- raster + gather, both windows first (keeps DVE streaming) ----
        for w in range(NWIN):
            ws = slice(w * WB, (w + 1) * WB)
            wsl = slice(w * WB * CAP, (w + 1) * WB * CAP)

            # 4 concurrent matmuls (row groups 0/32/64/96), 2 blocks each,
            # split into two 2-bank PSUM tiles so reduces pipeline
            for q in range(2):
                pw = ps.tile([128, WB * GCOLS // 2], fp32, tag="grid")
                for i in range(2):
                    g = 2 * q + i
                    t = w * WB + 2 * g
                    nc.tensor.matmul(
                        pw[:, i * 512:(i + 1) * 512],
                        lhsT=pb_t[32 * g:32 * g + 3, :],
                        rhs=rcb_t[32 * g:32 * g + 3,
                                  t * GCOLS:(t + 2) * GCOLS],
                        start=True, stop=True,
                        tile_position=(32 * g, 0),
                    )
                nc.vector.tensor_reduce(
                    nk[:, (2 * w + q) * 4 * CAP:(2 * w + q + 1) * 4 * CAP],
                    pw[:].rearrange("p (f v) -> p f v", v=4),
                    axis=mybir.AxisListType.X, op=AL.max, negate=True)
            nc.vector.tensor_reduce(
                nkmax[:, ws],
                nk[:, wsl].rearrange("p (t s) -> p t s", s=CAP),
                axis=mybir.AxisListType.X, op=AL.max)
            nc.vector.max_index(mi8[:, ws], nkmax[:, ws], nk[:, wsl])

            # one batched dma_gather per window.  Its index operand lives
            # int16-wrapped on partitions 0-15: idxw[pp, 8g+h] =
            # row(pixel 16h+pp, block w*8+g).  Build it with 8 one-hot
            # matmuls (PE moves data across partitions) + 1 strided copy.
            if ablate != "raster":
                if w == 0:
                    nc.vector.tensor_copy(mif[:, ws], mi8[:, ws])
                else:
                    nc.vector.tensor_scalar(mif[:, ws], mi8[:, ws],
                                            float(w * WB * CAP), None,
                                            AL.add)
                ip = ps.tile([128, WB * WB], fp32, tag="idxps")
                for h in range(8):
                    nc.tensor.matmul(
                        ip[:, h * WB:(h + 1) * WB],
                        lhsT=selm[:, h * 128:(h + 1) * 128],
                        rhs=mif[:, ws],
                        start=True, stop=True,
                    )
                nc.vector.tensor_copy(
                    idxw[:, w * 64:(w + 1) * 64].rearrange(
                        "pp (g h) -> pp g h", g=WB),
                    ip[:].rearrange("pp (h g) -> pp g h", h=WB))
                nc.gpsimd.dma_gather(
                    out_ap=gt[:, ws, :],
                    in_ap=ctab[:],
                    idxs_ap=idxw[:, w * 64:(w + 1) * 64],
                    num_idxs=WB * 128,
                    num_idxs_reg=WB * 128,
                    elem_size=SROW,
                )

        if ablate in ("raster", "gather"):
            acca = sm.tile([128, 1], fp32, tag="acc")
            nc.vector.scalar_tensor_tensor(
                diff[:, :, 0], nkmax[:], 1.0, nkmax[:], op0=AL.mult,
                op1=AL.mult, accum_out=acca[:])
            from concourse import bass_isa as _bisa
            lsba = sm.tile([128, 1], fp32, tag="lsb")
            nc.gpsimd.partition_all_reduce(lsba[:], acca[:], channels=128,
                                           reduce_op=_bisa.ReduceOp.add)
            nc.sync.dma_start(out=lossp[:], in_=lsba[0:1, :])

        for w in range(NWIN if ablate is None else 0):
            ws = slice(w * WB, (w + 1) * WB)

            # ---- winner barycentric u_i = clip01(A' + B*xs0 + C'*r) ----
            u3 = sm.tile([128, WB, 3], fp32, tag="u3")
            cof = gt[:, ws, CUBE:CUBE + 18].bitcast(fp32)
            Av = cof[:, :, 0:9:3]
            Bv = cof[:, :, 1:9:3]
            Cv = cof[:, :, 2:9:3]
            nc.vector.scalar_tensor_tensor(
                u3[:], Bv, xs0[:, 0:1], Av, op0=AL.mult, op1=AL.add)
            nc.vector.scalar_tensor_tensor(
                u3[:], Cv, rvec[:, 0:1], u3[:], op0=AL.mult, op1=AL.add)
            nc.vector.tensor_scalar(u3[:], u3[:], 0.0, 1.0, AL.max, AL.min)
            ssum = sm.tile([128, WB], fp32, tag="ssum")
            nc.vector.tensor_reduce(ssum[:], u3[:],
                                    axis=mybir.AxisListType.X, op=AL.add)
            nc.vector.tensor_scalar(ssum[:], ssum[:], 1e-8, None, AL.add)
            rcp = sm.tile([128, WB], fp32, tag="rcp")
            nc.vector.reciprocal(rcp[:], ssum[:])
            q3 = sm.tile([128, WB, 3], fp32, tag="q3")
            nc.vector.tensor_tensor(
                q3[:], u3[:],
                rcp[:].unsqueeze(2).broadcast_to((128, WB, 3)), op=AL.mult)

            # ---- tents for all 3 axes: te = relu(1 - |3q - k|), fp32 in,
            # bf16 out ----
            d4 = sb.tile([128, WB, 3, 4], fp32, tag="d4")
            tw = sb.tile([128, WB, 3, 4], fp32, tag="tw")
            te = sb.tile([128, WB, 3, 4], bf16, tag="te")
            nc.vector.scalar_tensor_tensor(
                d4[:].rearrange("p t c k -> p (t c) k"),
                q3[:].rearrange("p t c -> p (t c)").unsqueeze(2)
                    .broadcast_to((128, WB * 3, 4)),
                3.0, kk[:, 3 * w * WB:3 * (w + 1) * WB, :],
                op0=AL.mult, op1=AL.subtract)
            nc.vector.tensor_scalar(tw[:], d4[:], -1.0, 1.0, AL.mult, AL.add)
            nc.vector.tensor_scalar(d4[:], d4[:], 1.0, None, AL.add)
            nc.vector.tensor_tensor(d4[:], d4[:], tw[:], op=AL.min)
            nc.vector.tensor_scalar(te[:], d4[:], 0.0, None, AL.max)

            # ---- separable weights w64 = t0 x t1 x t2 ----
            w01 = sb.tile([128, WB, 4, 4], bf16, tag="w01")
            nc.vector.tensor_tensor(
                w01[:],
                te[:, :, 0, :].unsqueeze(3).broadcast_to((128, WB, 4, 4)),
                te[:, :, 1, :].unsqueeze(2).broadcast_to((128, WB, 4, 4)),
                op=AL.mult)
            w64 = sb.tile([128, WB, 16, 4], bf16, tag="w64")
            nc.vector.tensor_tensor(
                w64[:],
                w01[:].rearrange("p t a b -> p t (a b)").unsqueeze(3)
                    .broadcast_to((128, WB, 16, 4)),
                te[:, :, 2, :].unsqueeze(2).broadcast_to((128, WB, 16, 4)),
                op=AL.mult)

            # ---- cube contraction: one 2x bf16 mult + in-place tree sum ----
            mb = sb.tile([128, WB, 3, 64], bf16, tag="mb")
            nc.vector.tensor_tensor(
                mb[:], gt[:, ws, 0:CUBE].rearrange("p t (c s) -> p t c s",
                                                   s=64),
                w64[:].rearrange("p t a b -> p t (a b)").unsqueeze(2)
                    .broadcast_to((128, WB, 3, 64)),
                op=AL.mult)
            n = 64
            while n > 1:
                h = n // 2
                nc.vector.tensor_tensor(
                    mb[:, :, :, 0:h], mb[:, :, :, 0:h], mb[:, :, :, h:n],
                    op=AL.add)
                n = h

            # ---- hit mask + diff vs reference ----
            hm = sm.tile([128, WB], fp32, tag="hm")
            nc.vector.tensor_scalar(hm[:], nkmax[:, ws], -HIT_THRESH, None,
                                    AL.is_gt)
            flat = sm.tile([128, WB, 3], fp32, tag="flat")
            nc.vector.tensor_tensor(
                flat[:], mb[:, :, :, 0],
                hm[:].unsqueeze(2).broadcast_to((128, WB, 3)), op=AL.mult)
            nc.vector.tensor_tensor(diff[:, ws, :], flat[:], rs[:, ws, :],
                                    op=AL.subtract)

        # ---- squared-error accumulate (DVE) + partition reduce (Pool) ----
        if ablate is None:
            sq = sb.tile([128, NBLK * 3], fp32, tag="sq")
            acc = sm.tile([128, 1], fp32, tag="acc")
            dv = diff[:].rearrange("p t c -> p (t c)")
            nc.vector.scalar_tensor_tensor(sq[:], dv, 1.0, dv, op0=AL.mult,
                                           op1=AL.mult, accum_out=acc[:])
            from concourse import bass_isa
            lsb = sm.tile([128, 1], fp32, tag="lsb")
            nc.gpsimd.partition_all_reduce(lsb[:], acc[:], channels=128,
                                           reduce_op=bass_isa.ReduceOp.add)
            nc.sync.dma_start(out=lossp[:], in_=lsb[0:1, :])
        if probes:
            nc.sync.dma_start(out=p_nkmax[:], in_=nkmax[:])
            nc.sync.dma_start(out=p_mi8[:], in_=mi8[:])
            nc.sync.dma_start(out=p_gt[:],
                              in_=gt[:].rearrange("p t c -> p (t c)"))
            nc.sync.dma_start(out=p_diff[:],
                              in_=diff[:].rearrange("p t c -> p (t c)"))
            nc.sync.dma_start(out=p_acc[:], in_=acc[:])
            nc.sync.dma_start(out=p_nk[:], in_=nk[:])

    nc.compile()
    return nc


def _binned_in_maps(np_inputs, geom, lists):
    """Host tables for the v3 binned program."""
    w0c, w1c, w2c, Dc, valid = (geom["w0c"], geom["w1c"], geom["w2c"],
                                geom["Dc"], geom["valid"])
    xs = ((np.arange(W, dtype=np.float64) + 0.5) / W * 2.0 - 1.0)
    ys = (1.0 - (np.arange(H, dtype=np.float64) + 0.5) / H * 2.0)
    dy = -1.0 / 64.0
    nbc = W // BC

    # per-face coefficient stacks [3(basis rows A,B,C), F+1] with poison row
    def ext(c):
        z = np.zeros((3, F + 1))
        z[:, :F] = c
        return z
    e0, e1, e2, ed = ext(w0c), ext(w1c), ext(w2c), ext(Dc)

    # tanh'd texture cube, c-major, bf16 [F+1, 192] (as uint16 bit pattern)
    cube = np.tanh(np.asarray(np_inputs["textures"][0], np.float64))
    cube = cube.reshape(F, TS, TS, TS, 3).transpose(0, 4, 1, 2, 3)
    cube_ext = np.zeros((F + 1, CUBE), np.float32)
    cube_ext[:F] = cube.reshape(F, CUBE)
    cube_u16 = cube_ext.astype(ml_dtypes.bfloat16).view(np.uint16)

    pvalid = np.concatenate([valid, [False]])
    image_ref = np.asarray(np_inputs["image_ref"])

    # constant pixel basis, replicated into PE row groups 0/32/64/96
    j = np.arange(128) % 64
    r = (np.arange(128) // 64).astype(np.float64)
    xs0 = xs[j]
    pb3 = np.stack([np.ones(128), xs0, r]).astype(np.float32)  # [3,128]
    pb = np.zeros((99, 128), np.float32)
    for g in range(4):
        pb[32 * g:32 * g + 3] = pb3
    kk = np.broadcast_to(
        np.arange(4, dtype=np.float32),
        (128, NBLK * 3, 4)).reshape(128, NBLK * 12).copy()
    # one-hot selectors for the wrapped-index build: selm[p, 128h+j] =
    # (p == 16h + j%16) -- output replicated across 16-partition groups
    selm = np.zeros((128, WB * 128), np.float16)
    for h in range(WB):
        for j in range(128):
            selm[16 * h + j % 16, 128 * h + j] = 1.0

    in_maps = []
    for c in range(NCORES):
        li = lists[c]                                  # [NBLK, CAP]
        # per-block folded affine: A' = A + B*bj + C*py0 over basis (1,xs0,r)
        blkrow = np.arange(NBLK) // nbc
        bj = (np.arange(NBLK) % nbc).astype(np.float64)
        py0 = ys[c * TPC + blkrow * BR]
        rcb = np.zeros((3, NBLK, CAP, 4))
        cf = np.zeros((NBLK, CAP, CROW))
        for e_i, e in enumerate((e0, e1, e2)):
            A = e[0][li]; B = e[1][li]; C = e[2][li]   # [NBLK, CAP]
            Af = A + B * bj[:, None] + C * py0[:, None]
            rcb[0, :, :, e_i] = -KSCALE * Af
            rcb[1, :, :, e_i] = -KSCALE * B
            rcb[2, :, :, e_i] = -KSCALE * (C * dy)
            cf[:, :, 3 * e_i] = Af
            cf[:, :, 3 * e_i + 1] = B
            cf[:, :, 3 * e_i + 2] = C * dy
        A = ed[0][li]; B = ed[1][li]; C = ed[2][li]
        rcb[0, :, :, 3] = (A + B * bj[:, None] + C * py0[:, None]) + DSHIFT
        rcb[1, :, :, 3] = B
        rcb[2, :, :, 3] = C * dy
        # poison: padded slots and degenerate faces -> key 1e30
        poison = pvalid[li] == False                    # noqa: E712
        rcb[0][poison] = [1e30, 0.0, 0.0, 0.0]
        rcb[1][poison] = 0.0
        rcb[2][poison] = 0.0
        rcb = rcb.reshape(3, NBLK * CAP * 4).astype(np.float32)
        rcbq = np.zeros((99, NBLK * CAP * 4), np.float32)
        for g in range(4):
            rcbq[32 * g:32 * g + 3] = rcb

        # combined slot row: [192 bf16 cube | 9 fp32 coefs as 18 u16 | pad]
        ctab = np.zeros((NBLK * CAP, SROW), np.uint16)
        ctab[:, 0:CUBE] = cube_u16[li.reshape(-1)]
        cf32 = cf.reshape(NBLK * CAP, CROW).astype(np.float32)
        ctab[:, CUBE:CUBE + 18] = cf32[:, 0:9].copy().view(np.uint16)
        ctab = ctab.view(ml_dtypes.bfloat16)

        refsl = np.zeros((128, NBLK, 3), np.float32)
        for t in range(NBLK):
            br_, bj_ = divmod(t, nbc)
            rows = c * TPC + br_ * BR + np.arange(BR)
            cols = bj_ * BC + np.arange(BC)
            refsl[:, t, :] = image_ref[0][:, rows, :][:, :, cols] \
                .transpose(1, 2, 0).reshape(128, 3)

        in_maps.append({
            "rcb": rcbq, "ctab": ctab,
            "pb": pb, "xs0": pb3[1:2].T.copy(), "rvec": pb3[2:3].T.copy(),
            "kk": kk, "selm": selm, "refsl": refsl.reshape(128, NBLK * 3),
        })
    return in_maps


_last_exec_ns = None
_last_results = None
_last_in_maps = None


def kernel(vertices=None, textures=None, image_ref=None, faces=None,
           _trace=False, **kw):
    global _last_exec_ns, _last_results, _last_in_maps
    from concourse.bass_utils import run_bass_kernel_spmd

    vertices = np.asarray(vertices)
    textures = np.asarray(textures)
    image_ref = np.asarray(image_ref)
    faces = np.asarray(faces)
    np_inputs = {"vertices": vertices, "textures": textures,
                 "image_ref": image_ref, "faces": faces}

    geom = _geom(vertices, faces)
    lists = _bin_faces(geom)
    assert lists is not None, "bin overflow: CAP too small for this input"
    in_maps = _binned_in_maps(np_inputs, geom, lists)
    if "nc" not in _prog_cache:
        _prog_cache["nc"] = _build_binned()
    nc = _prog_cache["nc"]
    _last_in_maps = in_maps
    res = run_bass_kernel_spmd(nc, in_maps, core_ids=list(range(NCORES)),
                               trace=_trace)
    _last_exec_ns = res.exec_time_ns
    _last_results = res
    total = np.float32(0.0)
    for r in res.results:
        total += np.float32(r["lossp"].reshape(()))
    return np.asarray(total, np.float32)
